# revision 3
# baseline (speedup 1.0000x reference)
"""Trainium2 8-core fused attention kernel (QKV proj + RMSNorm + RoPE + SDPA + out proj).

Sharding: tensor-parallel over heads. Each of the 8 cores computes 2 of the 16
heads end-to-end (QKV projection with its Wqkv column shard, per-head RMSNorm +
RoPE, full softmax attention), then an AllToAll redistributes the per-head
attention outputs so every core holds all 1024 attention channels for 1/8 of
the tokens and applies the full Wout to its token shard.

v2: the softmax exp (the 950us baseline's pacer: 512 ACT instructions, ~590us)
is split across the Scalar engine (exact exp) and the Vector engine (Schraudolph
exp2 bit-trick straight into bf16 bits), score PSUM tiles are split per head so
the two engines work independently, stage-A elementwise work is rebalanced onto
ACT (Square + rinv-scaled PSUM->SBUF copies), q/k transposes moved from the PE
to the DMA xbar, and batch-0's AllToAll fires before batch-1's attention.

Self-contained: hardcodes all shapes from the problem spec.
"""
import os
import sys
import types

import numpy as np
import ml_dtypes

sys.path.insert(0, "/opt/trn_rl_repo")

from concourse import bass, bacc, tile, mybir  # noqa: E402
from concourse.bass_utils import run_bass_kernel_spmd  # noqa: E402

B, N, C, H, D = 2, 4096, 1024, 16, 64
NCORES = 8
TOK = B * N            # 8192 global tokens
NB = N // 128          # 32 token tiles per batch
NMACRO = N // 256      # 16 macro tiles (256 tok) per batch
QTILE = 512
NQT = N // QTILE       # 8 q tiles per batch
KC = N // 128          # 32 key chunks per batch
SHARD = TOK // NCORES  # 1024 tokens per core shard
EPS = 1e-6

F32 = mybir.dt.float32
F32R = mybir.dt.float32r
BF16 = mybir.dt.bfloat16
I16 = mybir.dt.int16
ALU = mybir.AluOpType
ACTF = mybir.ActivationFunctionType

# Schraudolph exp2 constants: bf16 bits of exp(0.125*s) ~= round(s*A + B)
SCH_A = float(0.125 * 128.0 / np.log(2.0))   # 23.0831...
SCH_B = float(127 * 128 - 5.5)

_CACHE = {}
_LAST_RESULT = None


def _install_profile_shim():
    """trn_boot skips the NTFF hook when antenv.axon_hooks is missing; supply it."""
    try:
        import antenv
        if getattr(antenv, "axon_hooks", None) is not None:
            return
        from trn_agent_boot.trn_boot import _ntff_profile_via_ctypes
        hook = _ntff_profile_via_ctypes("/opt/axon/libaxon_pjrt.so")
        if hook is None:
            return
        mod = types.ModuleType("antenv.axon_hooks")
        state = {"hook": hook}
        mod.get_axon_ntff_profile_hook = lambda: state["hook"]
        mod.set_axon_ntff_profile_hook = lambda h: state.__setitem__("hook", h)
        sys.modules["antenv.axon_hooks"] = mod
        antenv.axon_hooks = mod
    except Exception:
        pass


def _build_graph():
    nc = bacc.Bacc("TRN2", target_bir_lowering=False, debug=False,
                   enable_asserts=True, num_devices=NCORES)

    hsT_d = nc.dram_tensor("hsT", [C, TOK], BF16, kind="ExternalInput")
    wqkv_d = nc.dram_tensor("wqkv", [C, 384], BF16, kind="ExternalInput")
    trigc_d = nc.dram_tensor("trigc", [N, 256], BF16, kind="ExternalInput")
    trigs_d = nc.dram_tensor("trigs", [N, 256], BF16, kind="ExternalInput")
    wout_d = nc.dram_tensor("wout", [C, C], BF16, kind="ExternalInput")
    out_d = nc.dram_tensor("out", [SHARD, C], F32, kind="ExternalOutput")

    with tile.TileContext(nc) as tc:
        with tc.tile_pool(name="const", bufs=1) as constp, \
             tc.tile_pool(name="dram", bufs=1, space="DRAM") as dram:
            # resident weights
            wqkv_sb = constp.tile([128, 8, 384], BF16)
            nc.sync.dma_start(wqkv_sb[:], wqkv_d.ap().rearrange("(a p) n -> p a n", p=128))
            ones_f = constp.tile([65, 64], F32)
            nc.vector.memset(ones_f[:], 1.0)
            ones_sb = constp.tile([65, 64], F32R)
            nc.vector.tensor_copy(ones_sb[:], ones_f[:])

            a2a_in = [dram.tile([NCORES, 128, SHARD // 2], BF16,
                                  name=f"a2a_in{h}", tag=f"a2a_in{h}") for h in range(2)]
            a2a_out = [dram.tile([NCORES, 128, SHARD // 2], BF16,
                                   name=f"a2a_out{h}", tag=f"a2a_out{h}") for h in range(2)]

            with tc.tile_pool(name="batch", bufs=1) as bp, \
                 tc.tile_pool(name="stream", bufs=6) as sp, \
                 tc.tile_pool(name="work", bufs=3) as wp, \
                 tc.tile_pool(name="probsp", bufs=3) as pp, \
                 tc.tile_pool(name="ps1", bufs=2, space="PSUM") as ps1, \
                 tc.tile_pool(name="pssc", bufs=2, space="PSUM") as pssc, \
                 tc.tile_pool(name="psat", bufs=1, space="PSUM") as psat:

                qT = [bp.tile([128, N], BF16, name=f"qT{b}", tag=f"qT{b}") for b in range(B)]
                kT = [bp.tile([128, N], BF16, name=f"kT{b}", tag=f"kT{b}") for b in range(B)]
                vsb = [bp.tile([128, NB, 2, 65], BF16, name=f"v{b}", tag=f"v{b}")
                       for b in range(B)]
                atn = [[bp.tile([64, N], BF16, name=f"at{b}{h}", tag=f"at{b}{h}")
                        for h in range(2)] for b in range(B)]
                for b in range(B):
                    nc.vector.memset(vsb[b][:, :, :, 64:65], 1.0)

                # ------------- Stage A: QKV + RMSNorm + RoPE (per macro) -------
                def emit_A_load(b, mt):
                    """DMA loads for one macro tile (256 tokens)."""
                    hs_t = []
                    for cc in range(8):
                        t = sp.tile([128, 256], BF16, name=f"hs{cc}", tag=f"hs{cc}")
                        nc.sync.dma_start(
                            t[:], hsT_d.ap()[cc * 128:(cc + 1) * 128,
                                             b * N + mt * 256: b * N + (mt + 1) * 256])
                        hs_t.append(t)
                    trigC = sp.tile([128, 2, 256], BF16, name="trigC", tag="trigC")
                    trigS = sp.tile([128, 2, 256], BF16, name="trigS", tag="trigS")
                    for dst, dt_ in ((trigC, trigc_d), (trigS, trigs_d)):
                        nc.sync.dma_start(
                            dst[:], dt_.ap()[mt * 256:(mt + 1) * 256, :]
                            .rearrange("(s p) d -> p s d", p=128))
                    return hs_t, trigC, trigS

                def emit_A_sub(b, mt, sub, hs_t, trigC, trigS):
                    """QKV matmul + RMSNorm + RoPE for one 128-token sub tile."""
                    tt = mt * 2 + sub  # token tile index within batch
                    ps_qkv = ps1.tile([128, 384], F32, name="ps_qkv", tag="ps1")
                    for cc in range(8):
                        nc.tensor.matmul(
                            ps_qkv[:],
                            lhsT=hs_t[cc][:, sub * 128:(sub + 1) * 128],
                            rhs=wqkv_sb[:, cc, :],
                            start=(cc == 0), stop=(cc == 7))

                    # v block straight to SBUF (ACT, converts to bf16)
                    nc.scalar.activation(
                        vsb[b][:, tt, :, 0:64],
                        ps_qkv[:, 256:384].rearrange("p (h d) -> p h d", h=2),
                        ACTF.Copy)
                    # sum of squares per (q h0, q h1, k h0, k h1) group:
                    # ACT squares from PSUM, DVE reduces.
                    sq = wp.tile([128, 256], BF16, name="sq", tag="sq", bufs=3)
                    nc.scalar.activation(sq[:], ps_qkv[:, 0:256], ACTF.Square)
                    ssq4 = wp.tile([128, 4], F32, name="ssq4", tag="ssq4")
                    nc.vector.tensor_reduce(
                        ssq4[:], sq[:].rearrange("p (a e) -> p a e", a=4),
                        axis=mybir.AxisListType.X, op=ALU.add)
                    # rinv = 8/sqrt(ssq): bit-trick seed + 1 Newton step
                    # (the /64 mean and *8 fold together; eps negligible here)
                    yv = wp.tile([128, 4], F32, name="yv", tag="yv")
                    with nc.allow_low_precision(reason="rsqrt newton seed"):
                        nc.vector.tensor_scalar(
                            out=yv[:].bitcast(mybir.dt.int32),
                            in0=ssq4[:].bitcast(mybir.dt.int32),
                            scalar1=1, scalar2=None, op0=ALU.arith_shift_right)
                        nc.vector.tensor_scalar(
                            out=yv[:].bitcast(mybir.dt.int32),
                            in0=yv[:].bitcast(mybir.dt.int32),
                            scalar1=-1, scalar2=0x5F3759DF,
                            op0=ALU.mult, op1=ALU.add)
                    tn = wp.tile([128, 4], F32, name="tn", tag="tn")
                    nc.vector.tensor_mul(tn[:], yv[:], yv[:])
                    nc.vector.tensor_mul(tn[:], tn[:], ssq4[:])
                    nc.vector.tensor_scalar(out=tn[:], in0=tn[:],
                                            scalar1=-4.0, scalar2=12.0,
                                            op0=ALU.mult, op1=ALU.add)
                    nc.vector.tensor_mul(yv[:], yv[:], tn[:])
                    # normalized q/k in bf16 via per-partition-scaled ACT copies
                    qn = wp.tile([128, 256], BF16, name="qn", tag="qn", bufs=3)
                    for g in range(4):
                        nc.scalar.activation(
                            qn[:, g * 64:(g + 1) * 64],
                            ps_qkv[:, g * 64:(g + 1) * 64],
                            ACTF.Copy, scale=yv[:, g:g + 1])
                    # RoPE on normalized q/k (bf16 DVE ops)
                    d_qk = wp.tile([128, 256], BF16, name="d_qk", tag="d_qk", bufs=3)
                    nc.vector.tensor_mul(d_qk[:], qn[:], trigC[:, sub, :])
                    trot = wp.tile([128, 256], BF16, name="trot", tag="trot", bufs=3)
                    v4 = qn[:].rearrange("p (a e) -> p a e", a=8)
                    s4 = trigS[:, sub, :].rearrange("p (a e) -> p a e", a=8)
                    t4 = trot[:].rearrange("p (a e) -> p a e", a=8)
                    nc.vector.tensor_mul(t4[:, 0:8:2, :], v4[:, 1:8:2, :],
                                         s4[:, 0:8:2, :])
                    nc.vector.tensor_mul(t4[:, 1:8:2, :], v4[:, 0:8:2, :],
                                         s4[:, 1:8:2, :])
                    d_bf = wp.tile([128, 256], BF16, name="d_bf", tag="d_bf", bufs=3)
                    nc.vector.tensor_add(d_bf[:], d_qk[:], trot[:])
                    # q/k transposes via the DMA xbar (PE and DVE stay free)
                    nc.sync.dma_start(qT[b][:, tt * 128:(tt + 1) * 128],
                                      d_bf[:, 0:128], transpose=True)
                    nc.sync.dma_start(kT[b][:, tt * 128:(tt + 1) * 128],
                                      d_bf[:, 128:256], transpose=True)

                def emit_A(b, mt):
                    hs_t, trigC, trigS = emit_A_load(b, mt)
                    for sub in range(2):
                        emit_A_sub(b, mt, sub, hs_t, trigC, trigS)

                # ---------------- Stage B: attention --------------------------
                # at_acc: persistent accumulators for split-half q-tiles (batch 0)
                at_acc = [bp.tile([65, 2 * QTILE], F32R, name=f"at_acc{q}",
                                  tag=f"at_acc{q}") for q in range(NQT)]

                def emit_B(b, qt, kc_lo=0, kc_hi=KC - 1, acc=None,
                           act_extra=(), inter=None):
                    """Attention for one q tile over key chunks kc_lo..kc_hi.

                    Softmax exp: head 0 goes to the Scalar engine (exact exp),
                    head 1 to the Vector engine (Schraudolph bit-trick), except
                    kc in act_extra where head 1 also goes to Scalar.
                    inter: dict kc -> thunk emitted after that chunk (to
                    interleave stage-A work without starving the exp pipeline).
                    """
                    at_ps = psat.tile([65, 2 * QTILE], F32, name="at_ps", tag="psat",
                                      bufs=1)
                    at_ps_h = [at_ps[:, h * QTILE:(h + 1) * QTILE] for h in range(2)]
                    prev_pr = None
                    for kc in range(kc_lo, kc_hi + 1):
                        prs = []
                        for hh in range(2):
                            ps_s = pssc.tile([128, QTILE], F32, name=f"ps_s{hh}",
                                             tag=f"pssc{hh}")
                            nc.tensor.matmul(
                                ps_s[:],
                                lhsT=kT[b][64 * hh:64 * (hh + 1),
                                           kc * 128:(kc + 1) * 128],
                                rhs=qT[b][64 * hh:64 * (hh + 1),
                                          qt * QTILE:(qt + 1) * QTILE],
                                start=True, stop=True)
                            pr = pp.tile([128, QTILE], BF16, name=f"pr{hh}",
                                         tag=f"pr{hh}", bufs=3)
                            if hh == 0 or kc in act_extra:
                                nc.scalar.activation(pr[:], ps_s[:], ACTF.Exp,
                                                     bias=0.0, scale=0.125)
                            else:
                                with nc.allow_low_precision(reason="schraudolph exp"):
                                    nc.vector.tensor_scalar(
                                        out=pr[:].bitcast(I16),
                                        in0=ps_s[:],
                                        scalar1=SCH_A, scalar2=SCH_B,
                                        op0=ALU.mult, op1=ALU.add)
                            prs.append(pr)
                        if prev_pr is not None:
                            pkc, pprs = prev_pr
                            for hh in range(2):
                                nc.tensor.matmul(
                                    at_ps_h[hh],
                                    lhsT=vsb[b][:, pkc, hh, :],
                                    rhs=pprs[hh][:],
                                    start=(pkc == kc_lo), stop=(pkc == kc_hi))
                        prev_pr = (kc, prs)
                        if inter is not None and kc in inter:
                            inter[kc]()
                    pkc, pprs = prev_pr
                    for hh in range(2):
                        nc.tensor.matmul(
                            at_ps_h[hh],
                            lhsT=vsb[b][:, pkc, hh, :],
                            rhs=pprs[hh][:],
                            start=(pkc == kc_lo), stop=(pkc == kc_hi))
                    # evacuate attn psum to sbuf (frees psat for the next q-tile);
                    # for split q-tiles the evacuation doubles as accumulate.
                    if kc_hi < KC - 1:
                        # partial segment: accumulate and come back later
                        for hh in range(2):
                            dst = acc[:, hh * QTILE:(hh + 1) * QTILE]
                            if kc_lo == 0:
                                nc.vector.tensor_copy(dst, at_ps_h[hh])
                            else:
                                nc.vector.tensor_add(dst, dst, at_ps_h[hh])
                        return
                    if acc is not None and kc_lo > 0:
                        at_sb = acc
                        for hh in range(2):
                            dst = at_sb[:, hh * QTILE:(hh + 1) * QTILE]
                            nc.vector.tensor_add(dst, dst, at_ps_h[hh])
                    else:
                        at_sb = wp.tile([65, 2 * QTILE], F32R, name="at_sb", tag="at_sb",
                                        bufs=2)
                        for hh in range(2):
                            nc.vector.tensor_copy(at_sb[:, hh * QTILE:(hh + 1) * QTILE],
                                                  at_ps_h[hh])
                    for hh in range(2):
                        aps = at_sb[:, hh * QTILE:(hh + 1) * QTILE]
                        ps_bc = pssc.tile([64, QTILE], F32, name="ps_bc", tag="pssc0")
                        nc.tensor.matmul(
                            ps_bc[:],
                            lhsT=ones_sb[64:65, :],
                            rhs=aps[64:65, :],
                            start=True, stop=True)
                        rbc = wp.tile([64, QTILE], F32, name="rbc", tag="rbc", bufs=2)
                        nc.scalar.activation(rbc[:], ps_bc[:], ACTF.Copy)
                        rbcr = wp.tile([64, QTILE], F32, name="rbcr", tag="rbcr", bufs=2)
                        nc.vector.reciprocal_approx_fast(out=rbcr[:], in_=rbc[:])
                        nc.gpsimd.tensor_mul(
                            atn[b][hh][:, qt * QTILE:(qt + 1) * QTILE],
                            aps[0:64, :], rbcr[:])

                def stage_a2a(ha):
                    for dcore in range(NCORES):
                        base = dcore * 512
                        nc.sync.dma_start(a2a_in[ha][dcore, 0:64, :],
                                          atn[ha][0][:, base:base + 512])
                        nc.sync.dma_start(a2a_in[ha][dcore, 64:128, :],
                                          atn[ha][1][:, base:base + 512])

                def fire_a2a(ha):
                    nc.gpsimd.collective_compute(
                        "AllToAll", ALU.bypass,
                        ins=[a2a_in[ha][:].opt()], outs=[a2a_out[ha][:].opt()],
                        replica_groups=[list(range(NCORES))])

                # Emission order: lead-in A(b0 first half); P1: A(b0 rest)
                # interleaved with first-half attention of b0; P2: A(b1)
                # interleaved with second-half attention of b0; fire b0's
                # AllToAll; P3: b1 attention; fire b1's AllToAll.
                for mt in range(NMACRO // 2 + 1):
                    emit_A(0, mt)
                for qt in range(NQT):
                    # interleave macro 9+qt (its q tokens are first used by
                    # q-tile (9+qt)//2 >= qt+1, so emitting it inside this
                    # q-tile's FIFO order cannot deadlock)
                    mt = NMACRO // 2 + 1 + qt
                    if mt < NMACRO:
                        hs_t, trigC, trigS = emit_A_load(0, mt)
                        inter = {
                            3: (lambda a=(0, mt, 0, hs_t, trigC, trigS): emit_A_sub(*a)),
                            10: (lambda a=(0, mt, 1, hs_t, trigC, trigS): emit_A_sub(*a)),
                        }
                    else:
                        inter = None
                    emit_B(0, qt, 0, KC // 2 - 1, acc=at_acc[qt],
                           act_extra=(), inter=inter)
                for qt in range(NQT):
                    l0, l1 = emit_A_load(1, 2 * qt), emit_A_load(1, 2 * qt + 1)
                    inter = {
                        18: (lambda a=(1, 2 * qt, 0, *l0): emit_A_sub(*a)),
                        21: (lambda a=(1, 2 * qt, 1, *l0): emit_A_sub(*a)),
                        24: (lambda a=(1, 2 * qt + 1, 0, *l1): emit_A_sub(*a)),
                        27: (lambda a=(1, 2 * qt + 1, 1, *l1): emit_A_sub(*a)),
                    }
                    emit_B(0, qt, KC // 2, KC - 1, acc=at_acc[qt],
                           act_extra=(16, 24), inter=inter)
                stage_a2a(0)
                fire_a2a(0)
                for qt in range(NQT):
                    emit_B(1, qt, act_extra=(0, 16))
                stage_a2a(1)
            fire_a2a(1)

            # ---------------- Stage C: output projection ----------------------
            with tc.tile_pool(name="cstage", bufs=1) as cp, \
                 tc.tile_pool(name="cwork", bufs=2) as cw, \
                 tc.tile_pool(name="psC", bufs=2, space="PSUM") as psC:
                wout_sb = cp.tile([128, 8, C], BF16)
                nc.sync.dma_start(wout_sb[:], wout_d.ap().rearrange("(a p) n -> p a n", p=128))
                atf = cp.tile([128, 8, SHARD], BF16)
                for ha in range(2):
                    nc.sync.dma_start(atf[:, :, ha * 512:(ha + 1) * 512],
                                      a2a_out[ha][:].transpose([1, 0, 2]))
                    # b0's half of the projection starts while b1's AllToAll runs
                    for ttk in range(ha * 4, ha * 4 + 4):
                        ostage = cw.tile([128, C], F32, name="ostage", tag="ostage")
                        for half in range(2):
                            ps_o = psC.tile([128, 512], F32, name="ps_o", tag="psC")
                            for cc in range(8):
                                nc.tensor.matmul(
                                    ps_o[:],
                                    lhsT=atf[:, cc, ttk * 128:(ttk + 1) * 128],
                                    rhs=wout_sb[:, cc, half * 512:(half + 1) * 512],
                                    start=(cc == 0), stop=(cc == 7))
                            nc.vector.tensor_copy(ostage[:, half * 512:(half + 1) * 512], ps_o[:])
                        nc.sync.dma_start(out_d.ap()[ttk * 128:(ttk + 1) * 128, :], ostage[:])

    nc.compile()
    return nc


def _fold_sin(sin, g):
    out = np.empty_like(sin)
    out[:, :32] = -sin[:, :32] * g[32:]
    out[:, 32:] = sin[:, 32:] * g[:32]
    return out


def kernel(hidden_states, cos, sin, Wqkv, Wout, gq, gk):
    global _LAST_RESULT
    _install_profile_shim()

    hidden_states = np.asarray(hidden_states, dtype=np.float32)
    cos = np.asarray(cos, dtype=np.float32)
    sin = np.asarray(sin, dtype=np.float32)
    Wqkv = np.asarray(Wqkv, dtype=np.float32)
    Wout = np.asarray(Wout, dtype=np.float32)
    gq = np.asarray(gq, dtype=np.float32)
    gk = np.asarray(gk, dtype=np.float32)

    if "nc" not in _CACHE:
        _CACHE["nc"] = _build_graph()
    nc = _CACHE["nc"]

    hsT = np.ascontiguousarray(hidden_states.reshape(TOK, C).T).astype(ml_dtypes.bfloat16)
    cosq = cos * gq[None, :]
    sinq = _fold_sin(sin, gq)
    cosk = cos * gk[None, :]
    sink = _fold_sin(sin, gk)
    trigc = np.concatenate([cosq, cosq, cosk, cosk], axis=1).astype(ml_dtypes.bfloat16)
    trigs = np.concatenate([sinq, sinq, sink, sink], axis=1).astype(ml_dtypes.bfloat16)
    wout_bf = Wout.astype(ml_dtypes.bfloat16)

    in_maps = []
    for c in range(NCORES):
        wq = Wqkv[:, c * 128:(c + 1) * 128]
        wk = Wqkv[:, C + c * 128:C + (c + 1) * 128]
        wv = Wqkv[:, 2 * C + c * 128:2 * C + (c + 1) * 128]
        wqkv_loc = np.ascontiguousarray(
            np.concatenate([wq, wk, wv], axis=1)).astype(ml_dtypes.bfloat16)
        in_maps.append({
            "hsT": hsT, "wqkv": wqkv_loc, "trigc": trigc, "trigs": trigs,
            "wout": wout_bf,
        })

    trace = bool(os.environ.get("BASS_TRACE"))
    res = run_bass_kernel_spmd(nc, in_maps, core_ids=list(range(NCORES)), trace=trace)
    _LAST_RESULT = res

    full = np.empty((B, N, C), dtype=np.float32)
    for c in range(NCORES):
        o = res.results[c]["out"]
        for b in range(B):
            full[b, c * 512:(c + 1) * 512, :] = o[b * 512:(b + 1) * 512]
    return full


# revision 8
# speedup vs baseline: 1.1102x; 1.1102x over previous
"""Trainium2 8-core fused attention kernel (QKV proj + RMSNorm + RoPE + SDPA + out proj).

Sharding: tensor-parallel over heads. Each of the 8 cores computes 2 of the 16
heads end-to-end (QKV projection with its Wqkv column shard, per-head RMSNorm +
RoPE, full softmax attention), then an AllToAll redistributes the per-head
attention outputs so every core holds all 1024 attention channels for 1/8 of
the tokens and applies the full Wout to its token shard.

v2: the softmax exp (the 950us baseline's pacer: 512 ACT instructions, ~590us)
is split across the Scalar engine (exact exp) and the Vector engine (Schraudolph
exp2 bit-trick straight into bf16 bits), score PSUM tiles are split per head so
the two engines work independently, stage-A elementwise work is rebalanced onto
ACT (Square + rinv-scaled PSUM->SBUF copies), q/k transposes moved from the PE
to the DMA xbar, and batch-0's AllToAll fires before batch-1's attention.

Self-contained: hardcodes all shapes from the problem spec.
"""
import os
import sys
import types

import numpy as np
import ml_dtypes

sys.path.insert(0, "/opt/trn_rl_repo")

from concourse import bass, bacc, tile, mybir  # noqa: E402
from concourse.bass_utils import run_bass_kernel_spmd  # noqa: E402

B, N, C, H, D = 2, 4096, 1024, 16, 64
NCORES = 8
TOK = B * N            # 8192 global tokens
NB = N // 128          # 32 token tiles per batch
NMACRO = N // 256      # 16 macro tiles (256 tok) per batch
QTILE = 512
NQT = N // QTILE       # 8 q tiles per batch
KC = N // 128          # 32 key chunks per batch
SHARD = TOK // NCORES  # 1024 tokens per core shard
EPS = 1e-6

F32 = mybir.dt.float32
F32R = mybir.dt.float32r
BF16 = mybir.dt.bfloat16
I16 = mybir.dt.int16
ALU = mybir.AluOpType
ACTF = mybir.ActivationFunctionType

# Schraudolph exp2 constants: bf16 bits of exp(0.125*s) ~= round(s*A + B)
SCH_A = float(0.125 * 128.0 / np.log(2.0))   # 23.0831...
SCH_B = float(127 * 128 - 5.5)

_CACHE = {}
_LAST_RESULT = None


def _install_profile_shim():
    """trn_boot skips the NTFF hook when antenv.axon_hooks is missing; supply it."""
    try:
        import antenv
        if getattr(antenv, "axon_hooks", None) is not None:
            return
        from trn_agent_boot.trn_boot import _ntff_profile_via_ctypes
        hook = _ntff_profile_via_ctypes("/opt/axon/libaxon_pjrt.so")
        if hook is None:
            return
        mod = types.ModuleType("antenv.axon_hooks")
        state = {"hook": hook}
        mod.get_axon_ntff_profile_hook = lambda: state["hook"]
        mod.set_axon_ntff_profile_hook = lambda h: state.__setitem__("hook", h)
        sys.modules["antenv.axon_hooks"] = mod
        antenv.axon_hooks = mod
    except Exception:
        pass


def _build_graph():
    nc = bacc.Bacc("TRN2", target_bir_lowering=False, debug=False,
                   enable_asserts=True, num_devices=NCORES)

    hsT_d = nc.dram_tensor("hsT", [C, TOK], BF16, kind="ExternalInput")
    wqkv_d = nc.dram_tensor("wqkv", [C, 384], BF16, kind="ExternalInput")
    trigc_d = nc.dram_tensor("trigc", [N, 256], BF16, kind="ExternalInput")
    trigs_d = nc.dram_tensor("trigs", [N, 256], BF16, kind="ExternalInput")
    wout_d = nc.dram_tensor("wout", [C, C], BF16, kind="ExternalInput")
    out_d = nc.dram_tensor("out", [SHARD, C], F32, kind="ExternalOutput")

    with tile.TileContext(nc) as tc:
        with tc.tile_pool(name="const", bufs=1) as constp, \
             tc.tile_pool(name="dram", bufs=1, space="DRAM") as dram:
            # resident weights
            wqkv_sb = constp.tile([128, 8, 384], BF16)
            nc.sync.dma_start(wqkv_sb[:], wqkv_d.ap().rearrange("(a p) n -> p a n", p=128))
            ones_f = constp.tile([65, 64], F32)
            nc.vector.memset(ones_f[:], 1.0)
            ones_sb = constp.tile([65, 64], F32R)
            nc.vector.tensor_copy(ones_sb[:], ones_f[:])

            a2a_in = [dram.tile([NCORES, 128, SHARD // 2], BF16,
                                  name=f"a2a_in{h}", tag=f"a2a_in{h}") for h in range(2)]
            a2a_out = [dram.tile([NCORES, 128, SHARD // 2], BF16,
                                   name=f"a2a_out{h}", tag=f"a2a_out{h}") for h in range(2)]

            with tc.tile_pool(name="batch", bufs=1) as bp, \
                 tc.tile_pool(name="stream", bufs=6) as sp, \
                 tc.tile_pool(name="work", bufs=3) as wp, \
                 tc.tile_pool(name="probsp", bufs=3) as pp, \
                 tc.tile_pool(name="ps1", bufs=2, space="PSUM") as ps1, \
                 tc.tile_pool(name="pssc", bufs=2, space="PSUM") as pssc, \
                 tc.tile_pool(name="psat", bufs=1, space="PSUM") as psat:

                qT = [bp.tile([128, N], BF16, name=f"qT{b}", tag=f"qT{b}") for b in range(B)]
                kT = [bp.tile([128, N], BF16, name=f"kT{b}", tag=f"kT{b}") for b in range(B)]
                vsb = [bp.tile([128, NB, 2, 65], BF16, name=f"v{b}", tag=f"v{b}")
                       for b in range(B)]
                atn = [[bp.tile([64, N], BF16, name=f"at{b}{h}", tag=f"at{b}{h}")
                        for h in range(2)] for b in range(B)]
                for b in range(B):
                    nc.vector.memset(vsb[b][:, :, :, 64:65], 1.0)

                # ------------- Stage A: QKV + RMSNorm + RoPE (per macro) -------
                def emit_A_load(b, mt):
                    """DMA loads for one macro tile (256 tokens)."""
                    hs_t = sp.tile([128, 8, 256], BF16, name="hs", tag="hs")
                    nc.sync.dma_start(
                        hs_t[:], hsT_d.ap()[:, b * N + mt * 256: b * N + (mt + 1) * 256]
                        .rearrange("(a p) n -> p a n", p=128))
                    trigC = sp.tile([128, 2, 256], BF16, name="trigC", tag="trigC")
                    trigS = sp.tile([128, 2, 256], BF16, name="trigS", tag="trigS")
                    for dst, dt_ in ((trigC, trigc_d), (trigS, trigs_d)):
                        nc.sync.dma_start(
                            dst[:], dt_.ap()[mt * 256:(mt + 1) * 256, :]
                            .rearrange("(s p) d -> p s d", p=128))
                    return hs_t, trigC, trigS

                def emit_A_sub(b, mt, sub, hs_t, trigC, trigS):
                    """QKV matmul + RMSNorm + RoPE for one 128-token sub tile."""
                    tt = mt * 2 + sub  # token tile index within batch
                    ps_qkv = ps1.tile([128, 384], F32, name="ps_qkv", tag="ps1")
                    for cc in range(8):
                        nc.tensor.matmul(
                            ps_qkv[:],
                            lhsT=hs_t[:, cc, sub * 128:(sub + 1) * 128],
                            rhs=wqkv_sb[:, cc, :],
                            start=(cc == 0), stop=(cc == 7))

                    # v block straight to SBUF (ACT, converts to bf16)
                    nc.scalar.activation(
                        vsb[b][:, tt, :, 0:64],
                        ps_qkv[:, 256:384].rearrange("p (h d) -> p h d", h=2),
                        ACTF.Copy)
                    # sum of squares per (q h0, q h1, k h0, k h1) group:
                    # ACT squares from PSUM, DVE reduces.
                    sq = wp.tile([128, 256], BF16, name="sq", tag="sq", bufs=3)
                    nc.scalar.activation(sq[:], ps_qkv[:, 0:256], ACTF.Square)
                    ssq4 = wp.tile([128, 4], F32, name="ssq4", tag="ssq4")
                    nc.vector.tensor_reduce(
                        ssq4[:], sq[:].rearrange("p (a e) -> p a e", a=4),
                        axis=mybir.AxisListType.X, op=ALU.add)
                    # rinv = 8/sqrt(ssq): bit-trick seed + 1 Newton step
                    # (the /64 mean and *8 fold together; eps negligible here)
                    yv = wp.tile([128, 4], F32, name="yv", tag="yv")
                    with nc.allow_low_precision(reason="rsqrt newton seed"):
                        nc.vector.tensor_scalar(
                            out=yv[:].bitcast(mybir.dt.int32),
                            in0=ssq4[:].bitcast(mybir.dt.int32),
                            scalar1=1, scalar2=None, op0=ALU.arith_shift_right)
                        nc.vector.tensor_scalar(
                            out=yv[:].bitcast(mybir.dt.int32),
                            in0=yv[:].bitcast(mybir.dt.int32),
                            scalar1=-1, scalar2=0x5F3759DF,
                            op0=ALU.mult, op1=ALU.add)
                    tn = wp.tile([128, 4], F32, name="tn", tag="tn")
                    nc.vector.tensor_mul(tn[:], yv[:], yv[:])
                    nc.vector.tensor_mul(tn[:], tn[:], ssq4[:])
                    nc.vector.tensor_scalar(out=tn[:], in0=tn[:],
                                            scalar1=-4.0, scalar2=12.0,
                                            op0=ALU.mult, op1=ALU.add)
                    nc.vector.tensor_mul(yv[:], yv[:], tn[:])
                    # normalize all 4 groups straight out of PSUM (one DVE op)
                    qn = wp.tile([128, 256], BF16, name="qn", tag="qn", bufs=3)
                    nc.vector.tensor_tensor(
                        out=qn[:].rearrange("p (a e) -> p a e", a=4),
                        in0=ps_qkv[:, 0:256].rearrange("p (a e) -> p a e", a=4),
                        in1=yv[:].unsqueeze(2).broadcast_to([128, 4, 64]),
                        op=ALU.mult)
                    # RoPE on normalized q/k; the dense muls go to GpSimd (idle)
                    d_qk = wp.tile([128, 256], BF16, name="d_qk", tag="d_qk", bufs=3)
                    nc.gpsimd.tensor_mul(d_qk[:], qn[:], trigC[:, sub, :])
                    trot = wp.tile([128, 256], BF16, name="trot", tag="trot", bufs=3)
                    v4 = qn[:].rearrange("p (a e) -> p a e", a=8)
                    s4 = trigS[:, sub, :].rearrange("p (a e) -> p a e", a=8)
                    t4 = trot[:].rearrange("p (a e) -> p a e", a=8)
                    nc.vector.tensor_mul(t4[:, 0:8:2, :], v4[:, 1:8:2, :],
                                         s4[:, 0:8:2, :])
                    nc.vector.tensor_mul(t4[:, 1:8:2, :], v4[:, 0:8:2, :],
                                         s4[:, 1:8:2, :])
                    d_bf = wp.tile([128, 256], BF16, name="d_bf", tag="d_bf", bufs=3)
                    nc.gpsimd.tensor_add(d_bf[:], d_qk[:], trot[:])
                    # q/k transposes via the DMA xbar (PE and DVE stay free)
                    nc.sync.dma_start(qT[b][:, tt * 128:(tt + 1) * 128],
                                      d_bf[:, 0:128], transpose=True)
                    nc.sync.dma_start(kT[b][:, tt * 128:(tt + 1) * 128],
                                      d_bf[:, 128:256], transpose=True)

                def emit_A(b, mt):
                    hs_t, trigC, trigS = emit_A_load(b, mt)
                    for sub in range(2):
                        emit_A_sub(b, mt, sub, hs_t, trigC, trigS)

                # ---------------- Stage B: attention --------------------------
                # at_acc: persistent accumulators for split-half q-tiles (batch 0)
                at_acc = [bp.tile([65, 2 * QTILE], F32R, name=f"at_acc{q}",
                                  tag=f"at_acc{q}") for q in range(NQT)]

                def emit_B(b, qt, kc_lo=0, kc_hi=KC - 1, acc=None,
                           act_extra=(), inter=None, tail_prev=None):
                    """Attention for one q tile over key chunks kc_lo..kc_hi.

                    Softmax exp: head 0 goes to the Scalar engine (exact exp),
                    head 1 to the Vector engine (Schraudolph bit-trick), except
                    kc in act_extra where head 1 also goes to Scalar.
                    inter: dict kc -> thunk emitted after that chunk (to
                    interleave stage-A work without starving the exp pipeline).
                    tail_prev: the previous q-tile's deferred tail (final AV +
                    PSUM evacuation + epilogue); it is emitted after this
                    q-tile's second score/exp pair so the previous epilogue
                    drains while this q-tile's pipeline is already refilling.
                    Returns this q-tile's tail thunk.
                    """
                    at_ps = psat.tile([65, 2 * QTILE], F32, name="at_ps", tag="psat",
                                      bufs=1)
                    at_ps_h = [at_ps[:, h * QTILE:(h + 1) * QTILE] for h in range(2)]
                    prev_pr = None
                    for kc in range(kc_lo, kc_hi + 1):
                        prs = []
                        for hh in range(2):
                            ps_s = pssc.tile([128, QTILE], F32, name=f"ps_s{hh}",
                                             tag=f"pssc{hh}")
                            nc.tensor.matmul(
                                ps_s[:],
                                lhsT=kT[b][64 * hh:64 * (hh + 1),
                                           kc * 128:(kc + 1) * 128],
                                rhs=qT[b][64 * hh:64 * (hh + 1),
                                          qt * QTILE:(qt + 1) * QTILE],
                                start=True, stop=True)
                            pr = pp.tile([128, QTILE], BF16, name=f"pr{hh}",
                                         tag=f"pr{hh}", bufs=4)
                            if hh == 0 or kc in act_extra:
                                nc.scalar.activation(pr[:], ps_s[:], ACTF.Exp,
                                                     bias=0.0, scale=0.125)
                            else:
                                with nc.allow_low_precision(reason="schraudolph exp"):
                                    nc.vector.tensor_scalar(
                                        out=pr[:].bitcast(I16),
                                        in0=ps_s[:],
                                        scalar1=SCH_A, scalar2=SCH_B,
                                        op0=ALU.mult, op1=ALU.add)
                            prs.append(pr)
                        if kc == kc_lo + 1 and tail_prev is not None:
                            tail_prev()
                        if prev_pr is not None:
                            pkc, pprs = prev_pr
                            for hh in range(2):
                                nc.tensor.matmul(
                                    at_ps_h[hh],
                                    lhsT=vsb[b][:, pkc, hh, :],
                                    rhs=pprs[hh][:],
                                    start=(pkc == kc_lo), stop=(pkc == kc_hi))
                        prev_pr = (kc, prs)
                        if inter is not None and kc in inter:
                            inter[kc]()

                    def tail():
                        pkc, pprs = prev_pr
                        for hh in range(2):
                            nc.tensor.matmul(
                                at_ps_h[hh],
                                lhsT=vsb[b][:, pkc, hh, :],
                                rhs=pprs[hh][:],
                                start=(pkc == kc_lo), stop=(pkc == kc_hi))
                        # evacuate attn psum to sbuf (frees psat for the next
                        # q-tile); for split q-tiles it doubles as accumulate.
                        if kc_hi < KC - 1:
                            for hh in range(2):
                                dst = acc[:, hh * QTILE:(hh + 1) * QTILE]
                                if kc_lo == 0:
                                    nc.vector.tensor_copy(dst, at_ps_h[hh])
                                else:
                                    nc.vector.tensor_add(dst, dst, at_ps_h[hh])
                            return
                        if acc is not None and kc_lo > 0:
                            at_sb = acc
                            for hh in range(2):
                                dst = at_sb[:, hh * QTILE:(hh + 1) * QTILE]
                                nc.vector.tensor_add(dst, dst, at_ps_h[hh])
                        else:
                            at_sb = wp.tile([65, 2 * QTILE], F32R, name="at_sb",
                                            tag="at_sb", bufs=2)
                            for hh in range(2):
                                nc.vector.tensor_copy(
                                    at_sb[:, hh * QTILE:(hh + 1) * QTILE],
                                    at_ps_h[hh])
                        for hh in range(2):
                            aps = at_sb[:, hh * QTILE:(hh + 1) * QTILE]
                            ps_bc = pssc.tile([64, QTILE], F32, name="ps_bc",
                                              tag="pssc0")
                            nc.tensor.matmul(
                                ps_bc[:],
                                lhsT=ones_sb[64:65, :],
                                rhs=aps[64:65, :],
                                start=True, stop=True)
                            rbc = wp.tile([64, QTILE], F32, name="rbc", tag="rbc",
                                          bufs=2)
                            nc.scalar.activation(rbc[:], ps_bc[:], ACTF.Copy)
                            rbcr = wp.tile([64, QTILE], F32, name="rbcr", tag="rbcr",
                                           bufs=2)
                            nc.vector.reciprocal_approx_fast(out=rbcr[:], in_=rbc[:])
                            nc.gpsimd.tensor_mul(
                                atn[b][hh][:, qt * QTILE:(qt + 1) * QTILE],
                                aps[0:64, :], rbcr[:])
                    return tail

                def stage_a2a(ha):
                    for dcore in range(NCORES):
                        base = dcore * 512
                        nc.sync.dma_start(a2a_in[ha][dcore, 0:64, :],
                                          atn[ha][0][:, base:base + 512])
                        nc.sync.dma_start(a2a_in[ha][dcore, 64:128, :],
                                          atn[ha][1][:, base:base + 512])

                def fire_a2a(ha):
                    nc.gpsimd.collective_compute(
                        "AllToAll", ALU.bypass,
                        ins=[a2a_in[ha][:].opt()], outs=[a2a_out[ha][:].opt()],
                        replica_groups=[list(range(NCORES))])

                # Emission order: lead-in A(b0 first half); P1: A(b0 rest)
                # interleaved with first-half attention of b0; P2: A(b1)
                # interleaved with second-half attention of b0; fire b0's
                # AllToAll; P3: b1 attention; fire b1's AllToAll.
                tail = None
                for mt in range(NMACRO // 2 + 1):
                    emit_A(0, mt)
                for qt in range(NQT):
                    # interleave macro 9+qt (its q tokens are first used by
                    # q-tile (9+qt)//2 >= qt+1, so emitting it inside this
                    # q-tile's FIFO order cannot deadlock)
                    mt = NMACRO // 2 + 1 + qt
                    if mt < NMACRO:
                        a_args = emit_A_load(0, mt)
                        inter = {
                            3: (lambda a=(0, mt, 0, *a_args): emit_A_sub(*a)),
                            10: (lambda a=(0, mt, 1, *a_args): emit_A_sub(*a)),
                        }
                    else:
                        inter = None
                    tail = emit_B(0, qt, 0, KC // 2 - 1, acc=at_acc[qt],
                                  act_extra=(8,), inter=inter, tail_prev=tail)
                for qt in range(NQT):
                    l0, l1 = emit_A_load(1, 2 * qt), emit_A_load(1, 2 * qt + 1)
                    inter = {
                        18: (lambda a=(1, 2 * qt, 0, *l0): emit_A_sub(*a)),
                        21: (lambda a=(1, 2 * qt, 1, *l0): emit_A_sub(*a)),
                        24: (lambda a=(1, 2 * qt + 1, 0, *l1): emit_A_sub(*a)),
                        27: (lambda a=(1, 2 * qt + 1, 1, *l1): emit_A_sub(*a)),
                    }
                    tail = emit_B(0, qt, KC // 2, KC - 1, acc=at_acc[qt],
                                  act_extra=(20, 28), inter=inter, tail_prev=tail)
                for qt in range(NQT):
                    tail = emit_B(1, qt, act_extra=(8,), tail_prev=tail)
                    if qt == 0:
                        # b0's tail was just emitted inside this emit_B, so its
                        # AllToAll can go on the queues now and overlap the rest
                        # of b1's attention.
                        stage_a2a(0)
                        fire_a2a(0)
                tail()
                stage_a2a(1)
            fire_a2a(1)

            # ---------------- Stage C: output projection ----------------------
            with tc.tile_pool(name="cstage", bufs=1) as cp, \
                 tc.tile_pool(name="cwork", bufs=2) as cw, \
                 tc.tile_pool(name="psC", bufs=2, space="PSUM") as psC:
                wout_sb = cp.tile([128, 8, C], BF16)
                nc.sync.dma_start(wout_sb[:], wout_d.ap().rearrange("(a p) n -> p a n", p=128))
                atf = cp.tile([128, 8, SHARD], BF16)
                for ha in range(2):
                    nc.sync.dma_start(atf[:, :, ha * 512:(ha + 1) * 512],
                                      a2a_out[ha][:].transpose([1, 0, 2]))
                    # b0's half of the projection starts while b1's AllToAll runs
                    for ttk in range(ha * 4, ha * 4 + 4):
                        ostage = cw.tile([128, C], F32, name="ostage", tag="ostage")
                        for half in range(2):
                            ps_o = psC.tile([128, 512], F32, name="ps_o", tag="psC")
                            for cc in range(8):
                                nc.tensor.matmul(
                                    ps_o[:],
                                    lhsT=atf[:, cc, ttk * 128:(ttk + 1) * 128],
                                    rhs=wout_sb[:, cc, half * 512:(half + 1) * 512],
                                    start=(cc == 0), stop=(cc == 7))
                            nc.vector.tensor_copy(ostage[:, half * 512:(half + 1) * 512], ps_o[:])
                        nc.sync.dma_start(out_d.ap()[ttk * 128:(ttk + 1) * 128, :], ostage[:])

    nc.compile()
    return nc


def _fold_sin(sin, g):
    out = np.empty_like(sin)
    out[:, :32] = -sin[:, :32] * g[32:]
    out[:, 32:] = sin[:, 32:] * g[:32]
    return out


def kernel(hidden_states, cos, sin, Wqkv, Wout, gq, gk):
    global _LAST_RESULT
    _install_profile_shim()

    hidden_states = np.asarray(hidden_states, dtype=np.float32)
    cos = np.asarray(cos, dtype=np.float32)
    sin = np.asarray(sin, dtype=np.float32)
    Wqkv = np.asarray(Wqkv, dtype=np.float32)
    Wout = np.asarray(Wout, dtype=np.float32)
    gq = np.asarray(gq, dtype=np.float32)
    gk = np.asarray(gk, dtype=np.float32)

    if "nc" not in _CACHE:
        _CACHE["nc"] = _build_graph()
    nc = _CACHE["nc"]

    hsT = np.ascontiguousarray(hidden_states.reshape(TOK, C).T).astype(ml_dtypes.bfloat16)
    cosq = cos * gq[None, :]
    sinq = _fold_sin(sin, gq)
    cosk = cos * gk[None, :]
    sink = _fold_sin(sin, gk)
    trigc = np.concatenate([cosq, cosq, cosk, cosk], axis=1).astype(ml_dtypes.bfloat16)
    trigs = np.concatenate([sinq, sinq, sink, sink], axis=1).astype(ml_dtypes.bfloat16)
    wout_bf = Wout.astype(ml_dtypes.bfloat16)

    in_maps = []
    for c in range(NCORES):
        wq = Wqkv[:, c * 128:(c + 1) * 128]
        wk = Wqkv[:, C + c * 128:C + (c + 1) * 128]
        wv = Wqkv[:, 2 * C + c * 128:2 * C + (c + 1) * 128]
        wqkv_loc = np.ascontiguousarray(
            np.concatenate([wq, wk, wv], axis=1)).astype(ml_dtypes.bfloat16)
        in_maps.append({
            "hsT": hsT, "wqkv": wqkv_loc, "trigc": trigc, "trigs": trigs,
            "wout": wout_bf,
        })

    trace = bool(os.environ.get("BASS_TRACE"))
    res = run_bass_kernel_spmd(nc, in_maps, core_ids=list(range(NCORES)), trace=trace)
    _LAST_RESULT = res

    full = np.empty((B, N, C), dtype=np.float32)
    for c in range(NCORES):
        o = res.results[c]["out"]
        for b in range(B):
            full[b, c * 512:(c + 1) * 512, :] = o[b * 512:(b + 1) * 512]
    return full


# revision 22
# speedup vs baseline: 1.1167x; 1.0058x over previous
"""Trainium2 8-core fused attention kernel (QKV proj + RMSNorm + RoPE + SDPA + out proj).

Sharding: tensor-parallel over heads. Each of the 8 cores computes 2 of the 16
heads end-to-end (QKV projection with its Wqkv column shard, per-head RMSNorm +
RoPE, full softmax attention), then an AllToAll redistributes the per-head
attention outputs so every core holds all 1024 attention channels for 1/8 of
the tokens and applies the full Wout to its token shard.

v2: the softmax exp (the 950us baseline's pacer: 512 ACT instructions, ~590us)
is split across the Scalar engine (exact exp) and the Vector engine (Schraudolph
exp2 bit-trick straight into bf16 bits), score PSUM tiles are split per head so
the two engines work independently, stage-A elementwise work is rebalanced onto
ACT (Square + rinv-scaled PSUM->SBUF copies), q/k transposes moved from the PE
to the DMA xbar, and batch-0's AllToAll fires before batch-1's attention.

Self-contained: hardcodes all shapes from the problem spec.
"""
import os
import sys
import types

import numpy as np
import ml_dtypes

sys.path.insert(0, "/opt/trn_rl_repo")

from concourse import bass, bacc, tile, mybir  # noqa: E402
from concourse.bass_utils import run_bass_kernel_spmd  # noqa: E402

B, N, C, H, D = 2, 4096, 1024, 16, 64
NCORES = 8
TOK = B * N            # 8192 global tokens
NB = N // 128          # 32 token tiles per batch
NMACRO = N // 256      # 16 macro tiles (256 tok) per batch
QTILE = 512
NQT = N // QTILE       # 8 q tiles per batch
KC = N // 128          # 32 key chunks per batch
SHARD = TOK // NCORES  # 1024 tokens per core shard
EPS = 1e-6

F32 = mybir.dt.float32
F32R = mybir.dt.float32r
BF16 = mybir.dt.bfloat16
I16 = mybir.dt.int16
ALU = mybir.AluOpType
ACTF = mybir.ActivationFunctionType

# Schraudolph exp2 constants: bf16 bits of exp(0.125*s) ~= round(s*A + B)
SCH_A = float(0.125 * 128.0 / np.log(2.0))   # 23.0831...
SCH_B = float(127 * 128 - 5.5)

_CACHE = {}
_LAST_RESULT = None


def _install_profile_shim():
    """trn_boot skips the NTFF hook when antenv.axon_hooks is missing; supply it."""
    try:
        import antenv
        if getattr(antenv, "axon_hooks", None) is not None:
            return
        from trn_agent_boot.trn_boot import _ntff_profile_via_ctypes
        hook = _ntff_profile_via_ctypes("/opt/axon/libaxon_pjrt.so")
        if hook is None:
            return
        mod = types.ModuleType("antenv.axon_hooks")
        state = {"hook": hook}
        mod.get_axon_ntff_profile_hook = lambda: state["hook"]
        mod.set_axon_ntff_profile_hook = lambda h: state.__setitem__("hook", h)
        sys.modules["antenv.axon_hooks"] = mod
        antenv.axon_hooks = mod
    except Exception:
        pass


def _build_graph():
    nc = bacc.Bacc("TRN2", target_bir_lowering=False, debug=False,
                   enable_asserts=True, num_devices=NCORES)

    hsT_d = nc.dram_tensor("hsT", [C, TOK], BF16, kind="ExternalInput")
    wqkv_d = nc.dram_tensor("wqkv", [C, 384], BF16, kind="ExternalInput")
    trigc_d = nc.dram_tensor("trigc", [N, 256], BF16, kind="ExternalInput")
    trigs_d = nc.dram_tensor("trigs", [N, 256], BF16, kind="ExternalInput")
    wout_d = nc.dram_tensor("wout", [C, C], BF16, kind="ExternalInput")
    out_d = nc.dram_tensor("out", [SHARD, C], F32, kind="ExternalOutput")

    with tile.TileContext(nc) as tc:
        with tc.tile_pool(name="const", bufs=1) as constp, \
             tc.tile_pool(name="dram", bufs=1, space="DRAM") as dram:
            # resident weights
            wqkv_sb = constp.tile([128, 8, 384], BF16)
            nc.sync.dma_start(wqkv_sb[:], wqkv_d.ap().rearrange("(a p) n -> p a n", p=128))
            ones_f = constp.tile([65, 64], F32)
            nc.vector.memset(ones_f[:], 1.0)
            ones_sb = constp.tile([65, 64], F32R)
            nc.vector.tensor_copy(ones_sb[:], ones_f[:])
            wout_sb = constp.tile([128, 8, C], BF16)
            nc.sync.dma_start(wout_sb[:], wout_d.ap().rearrange("(a p) n -> p a n", p=128))
            atf = constp.tile([128, 8, SHARD], BF16)

            a2a_in = [dram.tile([NCORES, 128, SHARD // 2], BF16,
                                  name=f"a2a_in{h}", tag=f"a2a_in{h}") for h in range(2)]
            a2a_out = [dram.tile([NCORES, 128, SHARD // 2], BF16,
                                   name=f"a2a_out{h}", tag=f"a2a_out{h}") for h in range(2)]

            with tc.tile_pool(name="batch", bufs=1) as bp, \
                 tc.tile_pool(name="stream", bufs=4) as sp, \
                 tc.tile_pool(name="work", bufs=3) as wp, \
                 tc.tile_pool(name="probsp", bufs=3) as pp, \
                 tc.tile_pool(name="ps1", bufs=2, space="PSUM") as ps1, \
                 tc.tile_pool(name="pssc", bufs=2, space="PSUM") as pssc, \
                 tc.tile_pool(name="psat", bufs=1, space="PSUM") as psat:

                qT = [bp.tile([128, N], BF16, name=f"qT{b}", tag=f"qT{b}") for b in range(B)]
                kT = [bp.tile([128, N], BF16, name=f"kT{b}", tag=f"kT{b}") for b in range(B)]
                vsb = [bp.tile([128, NB, 2, 65], BF16, name=f"v{b}", tag=f"v{b}")
                       for b in range(B)]
                atn = [[bp.tile([64, N], BF16, name=f"at{b}{h}", tag=f"at{b}{h}")
                        for h in range(2)] for b in range(B)]
                for b in range(B):
                    nc.vector.memset(vsb[b][:, :, :, 64:65], 1.0)

                # ------------- Stage A: QKV + RMSNorm + RoPE (per macro) -------
                def emit_A_load(b, mt):
                    """DMA loads for one macro tile (256 tokens)."""
                    hs_t = sp.tile([128, 8, 256], BF16, name="hs", tag="hs")
                    nc.sync.dma_start(
                        hs_t[:], hsT_d.ap()[:, b * N + mt * 256: b * N + (mt + 1) * 256]
                        .rearrange("(a p) n -> p a n", p=128))
                    trigC = sp.tile([128, 2, 256], BF16, name="trigC", tag="trigC")
                    trigS = sp.tile([128, 2, 256], BF16, name="trigS", tag="trigS")
                    for dst, dt_ in ((trigC, trigc_d), (trigS, trigs_d)):
                        nc.sync.dma_start(
                            dst[:], dt_.ap()[mt * 256:(mt + 1) * 256, :]
                            .rearrange("(s p) d -> p s d", p=128))
                    return hs_t, trigC, trigS

                def emit_A_sub(b, mt, sub, hs_t, trigC, trigS):
                    """QKV matmul + RMSNorm + RoPE for one 128-token sub tile."""
                    tt = mt * 2 + sub  # token tile index within batch
                    ps_qkv = ps1.tile([128, 384], F32, name="ps_qkv", tag="ps1")
                    for cc in range(8):
                        nc.tensor.matmul(
                            ps_qkv[:],
                            lhsT=hs_t[:, cc, sub * 128:(sub + 1) * 128],
                            rhs=wqkv_sb[:, cc, :],
                            start=(cc == 0), stop=(cc == 7))

                    # v block straight to SBUF (ACT, converts to bf16)
                    nc.scalar.activation(
                        vsb[b][:, tt, :, 0:64],
                        ps_qkv[:, 256:384].rearrange("p (h d) -> p h d", h=2),
                        ACTF.Copy)
                    # sum of squares per (q h0, q h1, k h0, k h1) group:
                    # ACT squares from PSUM, DVE reduces.
                    sq = wp.tile([128, 256], BF16, name="sq", tag="sq", bufs=3)
                    nc.scalar.activation(sq[:], ps_qkv[:, 0:256], ACTF.Square)
                    ssq4 = wp.tile([128, 4], F32, name="ssq4", tag="ssq4")
                    nc.vector.tensor_reduce(
                        ssq4[:], sq[:].rearrange("p (a e) -> p a e", a=4),
                        axis=mybir.AxisListType.X, op=ALU.add)
                    # rinv = 8/sqrt(ssq): bit-trick seed + 1 Newton step
                    # (the /64 mean and *8 fold together; eps negligible here)
                    yv = wp.tile([128, 4], F32, name="yv", tag="yv")
                    with nc.allow_low_precision(reason="rsqrt newton seed"):
                        nc.vector.tensor_scalar(
                            out=yv[:].bitcast(mybir.dt.int32),
                            in0=ssq4[:].bitcast(mybir.dt.int32),
                            scalar1=1, scalar2=None, op0=ALU.arith_shift_right)
                        nc.vector.tensor_scalar(
                            out=yv[:].bitcast(mybir.dt.int32),
                            in0=yv[:].bitcast(mybir.dt.int32),
                            scalar1=-1, scalar2=0x5F3759DF,
                            op0=ALU.mult, op1=ALU.add)
                    tn = wp.tile([128, 4], F32, name="tn", tag="tn")
                    nc.vector.tensor_mul(tn[:], yv[:], yv[:])
                    nc.vector.tensor_mul(tn[:], tn[:], ssq4[:])
                    nc.vector.tensor_scalar(out=tn[:], in0=tn[:],
                                            scalar1=-4.0, scalar2=12.0,
                                            op0=ALU.mult, op1=ALU.add)
                    nc.vector.tensor_mul(yv[:], yv[:], tn[:])
                    # normalize all 4 groups straight out of PSUM (one DVE op)
                    qn = wp.tile([128, 256], BF16, name="qn", tag="qn", bufs=3)
                    nc.vector.tensor_tensor(
                        out=qn[:].rearrange("p (a e) -> p a e", a=4),
                        in0=ps_qkv[:, 0:256].rearrange("p (a e) -> p a e", a=4),
                        in1=yv[:].unsqueeze(2).broadcast_to([128, 4, 64]),
                        op=ALU.mult)
                    # RoPE on normalized q/k; the dense muls go to GpSimd (idle)
                    d_qk = wp.tile([128, 256], BF16, name="d_qk", tag="d_qk", bufs=3)
                    nc.gpsimd.tensor_mul(d_qk[:], qn[:], trigC[:, sub, :])
                    trot = wp.tile([128, 256], BF16, name="trot", tag="trot", bufs=3)
                    v4 = qn[:].rearrange("p (a e) -> p a e", a=8)
                    s4 = trigS[:, sub, :].rearrange("p (a e) -> p a e", a=8)
                    t4 = trot[:].rearrange("p (a e) -> p a e", a=8)
                    nc.gpsimd.tensor_mul(t4[:, 0:8:2, :], v4[:, 1:8:2, :],
                                         s4[:, 0:8:2, :])
                    nc.gpsimd.tensor_mul(t4[:, 1:8:2, :], v4[:, 0:8:2, :],
                                         s4[:, 1:8:2, :])
                    d_bf = wp.tile([128, 256], BF16, name="d_bf", tag="d_bf", bufs=3)
                    nc.gpsimd.tensor_add(d_bf[:], d_qk[:], trot[:])
                    # q/k transposes via the DMA xbar (PE and DVE stay free)
                    nc.sync.dma_start(qT[b][:, tt * 128:(tt + 1) * 128],
                                      d_bf[:, 0:128], transpose=True)
                    nc.sync.dma_start(kT[b][:, tt * 128:(tt + 1) * 128],
                                      d_bf[:, 128:256], transpose=True)

                def emit_A(b, mt):
                    hs_t, trigC, trigS = emit_A_load(b, mt)
                    for sub in range(2):
                        emit_A_sub(b, mt, sub, hs_t, trigC, trigS)

                # ---------------- Stage B: attention --------------------------
                # at_acc: persistent accumulators for split-half q-tiles (batch 0)
                at_acc = [bp.tile([65, 2 * QTILE], F32R, name=f"at_acc{q}",
                                  tag=f"at_acc{q}") for q in range(NQT)]

                def emit_B(b, qt, kc_lo=0, kc_hi=KC - 1, acc=None,
                           act_kcs=(), inter=None, tail_prev=None):
                    """Attention for one q tile over key chunks kc_lo..kc_hi.

                    Softmax exp per chunk is one wide [128,1024] instruction
                    covering both heads: on the Scalar engine (exact exp) for
                    kc in act_kcs, else on the Vector engine (Schraudolph
                    bit-trick straight to bf16 bits).
                    inter: dict kc -> thunk emitted after that chunk (to
                    interleave stage-A work without starving the exp pipeline).
                    tail_prev: the previous q-tile's deferred tail (final AV +
                    PSUM evacuation + epilogue); it is emitted after this
                    q-tile's second score/exp pair so the previous epilogue
                    drains while this q-tile's pipeline is already refilling.
                    Returns this q-tile's tail thunk.
                    """
                    at_ps = psat.tile([65, 2 * QTILE], F32, name="at_ps", tag="psat",
                                      bufs=1)
                    at_ps_h = [at_ps[:, h * QTILE:(h + 1) * QTILE] for h in range(2)]
                    pend = []

                    def emit_av(pkc, pr):
                        for hh in range(2):
                            nc.tensor.matmul(
                                at_ps_h[hh],
                                lhsT=vsb[b][:, pkc, hh, :],
                                rhs=pr[:, hh * QTILE:(hh + 1) * QTILE],
                                start=(pkc == kc_lo), stop=(pkc == kc_hi))

                    for kc in range(kc_lo, kc_hi + 1):
                        # both heads' scores into one PSUM tile so the whole
                        # chunk's softmax can be a single wide instruction
                        ps_s = pssc.tile([128, 2 * QTILE], F32, name="ps_s",
                                         tag="pssc")
                        for hh in range(2):
                            nc.tensor.matmul(
                                ps_s[:, hh * QTILE:(hh + 1) * QTILE],
                                lhsT=kT[b][64 * hh:64 * (hh + 1),
                                           kc * 128:(kc + 1) * 128],
                                rhs=qT[b][64 * hh:64 * (hh + 1),
                                          qt * QTILE:(qt + 1) * QTILE],
                                start=True, stop=True)
                        pr = pp.tile([128, 2 * QTILE], BF16, name="pr",
                                     tag="pr", bufs=4)
                        if kc in act_kcs:
                            nc.scalar.activation(pr[:], ps_s[:], ACTF.Exp,
                                                 bias=0.0, scale=0.125)
                        else:
                            with nc.allow_low_precision(reason="schraudolph exp"):
                                nc.vector.tensor_scalar(
                                    out=pr[:].bitcast(I16),
                                    in0=ps_s[:],
                                    scalar1=SCH_A, scalar2=SCH_B,
                                    op0=ALU.mult, op1=ALU.add)
                        if kc == kc_lo + 1 and tail_prev is not None:
                            tail_prev()
                        # AV lags the scores by 2 chunks so the PE FIFO always
                        # has the next score pair ahead of an exp-gated AV.
                        pend.append((kc, pr))
                        if len(pend) > 2:
                            emit_av(*pend.pop(0))
                        if inter is not None and kc in inter:
                            inter[kc]()

                    def tail():
                        for item in pend:
                            emit_av(*item)
                        # evacuate attn psum to sbuf (frees psat for the next
                        # q-tile); for split q-tiles it doubles as accumulate.
                        if kc_hi < KC - 1:
                            for hh in range(2):
                                dst = acc[:, hh * QTILE:(hh + 1) * QTILE]
                                if kc_lo == 0:
                                    nc.vector.tensor_copy(dst, at_ps_h[hh])
                                else:
                                    nc.vector.tensor_add(dst, dst, at_ps_h[hh])
                            return
                        if acc is not None and kc_lo > 0:
                            at_sb = acc
                            for hh in range(2):
                                dst = at_sb[:, hh * QTILE:(hh + 1) * QTILE]
                                nc.vector.tensor_add(dst, dst, at_ps_h[hh])
                        else:
                            at_sb = wp.tile([65, 2 * QTILE], F32R, name="at_sb",
                                            tag="at_sb", bufs=2)
                            for hh in range(2):
                                nc.vector.tensor_copy(
                                    at_sb[:, hh * QTILE:(hh + 1) * QTILE],
                                    at_ps_h[hh])
                        for hh in range(2):
                            aps = at_sb[:, hh * QTILE:(hh + 1) * QTILE]
                            ps_bc = pssc.tile([64, QTILE], F32, name="ps_bc",
                                              tag="pssc")
                            nc.tensor.matmul(
                                ps_bc[:],
                                lhsT=ones_sb[64:65, :],
                                rhs=aps[64:65, :],
                                start=True, stop=True)
                            rbc = wp.tile([64, QTILE], F32, name="rbc", tag="rbc",
                                          bufs=2)
                            nc.scalar.activation(rbc[:], ps_bc[:], ACTF.Copy)
                            rbcr = wp.tile([64, QTILE], F32, name="rbcr", tag="rbcr",
                                           bufs=2)
                            nc.vector.reciprocal_approx_fast(out=rbcr[:], in_=rbc[:])
                            nc.gpsimd.tensor_mul(
                                atn[b][hh][:, qt * QTILE:(qt + 1) * QTILE],
                                aps[0:64, :], rbcr[:])
                    return tail

                def stage_a2a(ha):
                    for dcore in range(NCORES):
                        base = dcore * 512
                        nc.sync.dma_start(a2a_in[ha][dcore, 0:64, :],
                                          atn[ha][0][:, base:base + 512])
                        nc.sync.dma_start(a2a_in[ha][dcore, 64:128, :],
                                          atn[ha][1][:, base:base + 512])

                def fire_a2a(ha):
                    nc.gpsimd.collective_compute(
                        "AllToAll", ALU.bypass,
                        ins=[a2a_in[ha][:].opt()], outs=[a2a_out[ha][:].opt()],
                        replica_groups=[list(range(NCORES))])

                # Emission order: lead-in A(b0 first half); P1: A(b0 rest)
                # interleaved with first-half attention of b0; P2: A(b1)
                # interleaved with second-half attention of b0; fire b0's
                # AllToAll; P3: b1 attention; fire b1's AllToAll.
                tail = None
                for mt in range(NMACRO // 2 + 1):
                    emit_A(0, mt)
                for qt in range(NQT):
                    # interleave macro 9+qt (its q tokens are first used by
                    # q-tile (9+qt)//2 >= qt+1, so emitting it inside this
                    # q-tile's FIFO order cannot deadlock)
                    mt = NMACRO // 2 + 1 + qt
                    if mt < NMACRO:
                        a_args = emit_A_load(0, mt)
                        inter = {
                            3: (lambda a=(0, mt, 0, *a_args): emit_A_sub(*a)),
                            10: (lambda a=(0, mt, 1, *a_args): emit_A_sub(*a)),
                        }
                    else:
                        inter = None
                    tail = emit_B(0, qt, 0, KC // 2 - 1, acc=at_acc[qt],
                                  act_kcs=(0, 2, 4, 6, 8, 10, 12, 14, 15),
                                  inter=inter, tail_prev=tail)
                for qt in range(NQT):
                    l0, l1 = emit_A_load(1, 2 * qt), emit_A_load(1, 2 * qt + 1)
                    inter = {
                        18: (lambda a=(1, 2 * qt, 0, *l0): emit_A_sub(*a)),
                        21: (lambda a=(1, 2 * qt, 1, *l0): emit_A_sub(*a)),
                        24: (lambda a=(1, 2 * qt + 1, 0, *l1): emit_A_sub(*a)),
                        27: (lambda a=(1, 2 * qt + 1, 1, *l1): emit_A_sub(*a)),
                    }
                    tail = emit_B(0, qt, KC // 2, KC - 1, acc=at_acc[qt],
                                  act_kcs=(16, 18, 20, 22, 24, 26, 28, 30, 31),
                                  inter=inter, tail_prev=tail)
                for qt in range(NQT):
                    tail = emit_B(1, qt,
                                  act_kcs=tuple(range(0, 32, 2)) + (31,),
                                  tail_prev=tail)
                    if qt == 0:
                        # b0's tail was just emitted inside this emit_B, so its
                        # AllToAll can go on the queues now and overlap the rest
                        # of b1's attention.
                        stage_a2a(0)
                        fire_a2a(0)
                        nc.sync.dma_start(atf[:, :, 0:512],
                                          a2a_out[0][:].transpose([1, 0, 2]))
                tail()
                stage_a2a(1)
            fire_a2a(1)
            nc.sync.dma_start(atf[:, :, 512:1024],
                              a2a_out[1][:].transpose([1, 0, 2]))

            # ---------------- Stage C: output projection ----------------------
            with tc.tile_pool(name="cwork", bufs=2) as cw, \
                 tc.tile_pool(name="psC", bufs=2, space="PSUM") as psC:
                for ttk in range(SHARD // 128):
                    ostage = cw.tile([128, C], F32, name="ostage", tag="ostage")
                    for half in range(2):
                        ps_o = psC.tile([128, 512], F32, name="ps_o", tag="psC")
                        for cc in range(8):
                            nc.tensor.matmul(
                                ps_o[:],
                                lhsT=atf[:, cc, ttk * 128:(ttk + 1) * 128],
                                rhs=wout_sb[:, cc, half * 512:(half + 1) * 512],
                                start=(cc == 0), stop=(cc == 7))
                        nc.vector.tensor_copy(ostage[:, half * 512:(half + 1) * 512], ps_o[:])
                    nc.sync.dma_start(out_d.ap()[ttk * 128:(ttk + 1) * 128, :], ostage[:])

    nc.compile()
    return nc


def _fold_sin(sin, g):
    out = np.empty_like(sin)
    out[:, :32] = -sin[:, :32] * g[32:]
    out[:, 32:] = sin[:, 32:] * g[:32]
    return out


def kernel(hidden_states, cos, sin, Wqkv, Wout, gq, gk):
    global _LAST_RESULT
    _install_profile_shim()

    hidden_states = np.asarray(hidden_states, dtype=np.float32)
    cos = np.asarray(cos, dtype=np.float32)
    sin = np.asarray(sin, dtype=np.float32)
    Wqkv = np.asarray(Wqkv, dtype=np.float32)
    Wout = np.asarray(Wout, dtype=np.float32)
    gq = np.asarray(gq, dtype=np.float32)
    gk = np.asarray(gk, dtype=np.float32)

    if "nc" not in _CACHE:
        _CACHE["nc"] = _build_graph()
    nc = _CACHE["nc"]

    hsT = np.ascontiguousarray(hidden_states.reshape(TOK, C).T).astype(ml_dtypes.bfloat16)
    cosq = cos * gq[None, :]
    sinq = _fold_sin(sin, gq)
    cosk = cos * gk[None, :]
    sink = _fold_sin(sin, gk)
    trigc = np.concatenate([cosq, cosq, cosk, cosk], axis=1).astype(ml_dtypes.bfloat16)
    trigs = np.concatenate([sinq, sinq, sink, sink], axis=1).astype(ml_dtypes.bfloat16)
    wout_bf = Wout.astype(ml_dtypes.bfloat16)

    in_maps = []
    for c in range(NCORES):
        wq = Wqkv[:, c * 128:(c + 1) * 128]
        wk = Wqkv[:, C + c * 128:C + (c + 1) * 128]
        wv = Wqkv[:, 2 * C + c * 128:2 * C + (c + 1) * 128]
        wqkv_loc = np.ascontiguousarray(
            np.concatenate([wq, wk, wv], axis=1)).astype(ml_dtypes.bfloat16)
        in_maps.append({
            "hsT": hsT, "wqkv": wqkv_loc, "trigc": trigc, "trigs": trigs,
            "wout": wout_bf,
        })

    trace = bool(os.environ.get("BASS_TRACE"))
    res = run_bass_kernel_spmd(nc, in_maps, core_ids=list(range(NCORES)), trace=trace)
    _LAST_RESULT = res

    full = np.empty((B, N, C), dtype=np.float32)
    for c in range(NCORES):
        o = res.results[c]["out"]
        for b in range(B):
            full[b, c * 512:(c + 1) * 512, :] = o[b * 512:(b + 1) * 512]
    return full


# revision 24
# speedup vs baseline: 1.1455x; 1.0258x over previous
"""Trainium2 8-core fused attention kernel (QKV proj + RMSNorm + RoPE + SDPA + out proj).

Sharding: tensor-parallel over heads. Each of the 8 cores computes 2 of the 16
heads end-to-end (QKV projection with its Wqkv column shard, per-head RMSNorm +
RoPE, full softmax attention), then an AllToAll redistributes the per-head
attention outputs so every core holds all 1024 attention channels for 1/8 of
the tokens and applies the full Wout to its token shard.

v2: the softmax exp (the 950us baseline's pacer: 512 ACT instructions, ~590us)
is split across the Scalar engine (exact exp) and the Vector engine (Schraudolph
exp2 bit-trick straight into bf16 bits), score PSUM tiles are split per head so
the two engines work independently, stage-A elementwise work is rebalanced onto
ACT (Square + rinv-scaled PSUM->SBUF copies), q/k transposes moved from the PE
to the DMA xbar, and batch-0's AllToAll fires before batch-1's attention.

Self-contained: hardcodes all shapes from the problem spec.
"""
import os
import sys
import types

import numpy as np
import ml_dtypes

sys.path.insert(0, "/opt/trn_rl_repo")

from concourse import bass, bacc, tile, mybir  # noqa: E402
from concourse.bass_utils import run_bass_kernel_spmd  # noqa: E402

B, N, C, H, D = 2, 4096, 1024, 16, 64
NCORES = 8
TOK = B * N            # 8192 global tokens
NB = N // 128          # 32 token tiles per batch
NMACRO = N // 256      # 16 macro tiles (256 tok) per batch
QTILE = 512
NQT = N // QTILE       # 8 q tiles per batch
KC = N // 128          # 32 key chunks per batch
SHARD = TOK // NCORES  # 1024 tokens per core shard
EPS = 1e-6

F32 = mybir.dt.float32
F32R = mybir.dt.float32r
BF16 = mybir.dt.bfloat16
I16 = mybir.dt.int16
ALU = mybir.AluOpType
ACTF = mybir.ActivationFunctionType

# Schraudolph exp2 constants: bf16 bits of exp(0.125*s) ~= round(s*A + B)
SCH_A = float(0.125 * 128.0 / np.log(2.0))   # 23.0831...
SCH_B = float(127 * 128 - 5.5)

_CACHE = {}
_LAST_RESULT = None


def _install_profile_shim():
    """trn_boot skips the NTFF hook when antenv.axon_hooks is missing; supply it."""
    try:
        import antenv
        if getattr(antenv, "axon_hooks", None) is not None:
            return
        from trn_agent_boot.trn_boot import _ntff_profile_via_ctypes
        hook = _ntff_profile_via_ctypes("/opt/axon/libaxon_pjrt.so")
        if hook is None:
            return
        mod = types.ModuleType("antenv.axon_hooks")
        state = {"hook": hook}
        mod.get_axon_ntff_profile_hook = lambda: state["hook"]
        mod.set_axon_ntff_profile_hook = lambda h: state.__setitem__("hook", h)
        sys.modules["antenv.axon_hooks"] = mod
        antenv.axon_hooks = mod
    except Exception:
        pass


def _build_graph():
    nc = bacc.Bacc("TRN2", target_bir_lowering=False, debug=False,
                   enable_asserts=True, num_devices=NCORES)

    hsT_d = nc.dram_tensor("hsT", [C, TOK], BF16, kind="ExternalInput")
    wqkv_d = nc.dram_tensor("wqkv", [C, 384], BF16, kind="ExternalInput")
    trigc_d = nc.dram_tensor("trigc", [N, 256], BF16, kind="ExternalInput")
    trigs_d = nc.dram_tensor("trigs", [N, 256], BF16, kind="ExternalInput")
    wout_d = nc.dram_tensor("wout", [C, C], BF16, kind="ExternalInput")
    out_d = nc.dram_tensor("out", [SHARD, C], F32, kind="ExternalOutput")

    with tile.TileContext(nc) as tc:
        with tc.tile_pool(name="const", bufs=1) as constp, \
             tc.tile_pool(name="dram", bufs=1, space="DRAM") as dram:
            # resident weights
            wqkv_sb = constp.tile([128, 8, 384], BF16)
            nc.sync.dma_start(wqkv_sb[:], wqkv_d.ap().rearrange("(a p) n -> p a n", p=128))
            ones_f = constp.tile([65, 64], F32)
            nc.vector.memset(ones_f[:], 1.0)
            ones_sb = constp.tile([65, 64], F32R)
            nc.vector.tensor_copy(ones_sb[:], ones_f[:])
            wout_sb = constp.tile([128, 8, C], BF16)
            nc.sync.dma_start(wout_sb[:], wout_d.ap().rearrange("(a p) n -> p a n", p=128))
            atf = constp.tile([128, 8, SHARD], BF16)

            a2a_in = [dram.tile([NCORES, 128, SHARD // 2], BF16,
                                  name=f"a2a_in{h}", tag=f"a2a_in{h}") for h in range(2)]
            a2a_out = [dram.tile([NCORES, 128, SHARD // 2], BF16,
                                   name=f"a2a_out{h}", tag=f"a2a_out{h}") for h in range(2)]

            with tc.tile_pool(name="batch", bufs=1) as bp, \
                 tc.tile_pool(name="stream", bufs=4) as sp, \
                 tc.tile_pool(name="work", bufs=3) as wp, \
                 tc.tile_pool(name="probsp", bufs=3) as pp, \
                 tc.tile_pool(name="pssc", bufs=3, space="PSUM") as pssc, \
                 tc.tile_pool(name="psat", bufs=1, space="PSUM") as psat:

                qT = [bp.tile([128, N], BF16, name=f"qT{b}", tag=f"qT{b}") for b in range(B)]
                kT = [bp.tile([128, N], BF16, name=f"kT{b}", tag=f"kT{b}") for b in range(B)]
                vsb = [bp.tile([128, NB, 2, 65], BF16, name=f"v{b}", tag=f"v{b}")
                       for b in range(B)]
                atn = [[bp.tile([64, N], BF16, name=f"at{b}{h}", tag=f"at{b}{h}")
                        for h in range(2)] for b in range(B)]
                for b in range(B):
                    nc.vector.memset(vsb[b][:, :, :, 64:65], 1.0)

                # ------------- Stage A: QKV + RMSNorm + RoPE (per macro) -------
                def emit_A_load(b, mt):
                    """DMA loads for one macro tile (256 tokens)."""
                    hs_t = sp.tile([128, 8, 256], BF16, name="hs", tag="hs")
                    nc.sync.dma_start(
                        hs_t[:], hsT_d.ap()[:, b * N + mt * 256: b * N + (mt + 1) * 256]
                        .rearrange("(a p) n -> p a n", p=128))
                    trigC = sp.tile([128, 2, 256], BF16, name="trigC", tag="trigC")
                    trigS = sp.tile([128, 2, 256], BF16, name="trigS", tag="trigS")
                    for dst, dt_ in ((trigC, trigc_d), (trigS, trigs_d)):
                        nc.sync.dma_start(
                            dst[:], dt_.ap()[mt * 256:(mt + 1) * 256, :]
                            .rearrange("(s p) d -> p s d", p=128))
                    return hs_t, trigC, trigS

                def emit_A_sub(b, mt, sub, hs_t, trigC, trigS):
                    """QKV matmul + RMSNorm + RoPE for one 128-token sub tile."""
                    tt = mt * 2 + sub  # token tile index within batch
                    # ps_qkv shares the score-tile PSUM ring (the extra ring
                    # depth is what lets scores run 3 chunks ahead of exp)
                    ps_qkv = pssc.tile([128, 384], F32, name="ps_qkv", tag="pssc")
                    for cc in range(8):
                        nc.tensor.matmul(
                            ps_qkv[:],
                            lhsT=hs_t[:, cc, sub * 128:(sub + 1) * 128],
                            rhs=wqkv_sb[:, cc, :],
                            start=(cc == 0), stop=(cc == 7))

                    # v block straight to SBUF (ACT, converts to bf16)
                    nc.scalar.activation(
                        vsb[b][:, tt, :, 0:64],
                        ps_qkv[:, 256:384].rearrange("p (h d) -> p h d", h=2),
                        ACTF.Copy)
                    # sum of squares per (q h0, q h1, k h0, k h1) group:
                    # ACT squares from PSUM, DVE reduces.
                    sq = wp.tile([128, 256], BF16, name="sq", tag="sq", bufs=3)
                    nc.scalar.activation(sq[:], ps_qkv[:, 0:256], ACTF.Square)
                    ssq4 = wp.tile([128, 4], F32, name="ssq4", tag="ssq4")
                    nc.vector.tensor_reduce(
                        ssq4[:], sq[:].rearrange("p (a e) -> p a e", a=4),
                        axis=mybir.AxisListType.X, op=ALU.add)
                    # rinv = 8/sqrt(ssq): bit-trick seed + 1 Newton step
                    # (the /64 mean and *8 fold together; eps negligible here)
                    yv = wp.tile([128, 4], F32, name="yv", tag="yv")
                    with nc.allow_low_precision(reason="rsqrt newton seed"):
                        nc.vector.tensor_scalar(
                            out=yv[:].bitcast(mybir.dt.int32),
                            in0=ssq4[:].bitcast(mybir.dt.int32),
                            scalar1=1, scalar2=None, op0=ALU.arith_shift_right)
                        nc.vector.tensor_scalar(
                            out=yv[:].bitcast(mybir.dt.int32),
                            in0=yv[:].bitcast(mybir.dt.int32),
                            scalar1=-1, scalar2=0x5F3759DF,
                            op0=ALU.mult, op1=ALU.add)
                    tn = wp.tile([128, 4], F32, name="tn", tag="tn")
                    nc.vector.tensor_mul(tn[:], yv[:], yv[:])
                    nc.vector.tensor_mul(tn[:], tn[:], ssq4[:])
                    nc.vector.tensor_scalar(out=tn[:], in0=tn[:],
                                            scalar1=-4.0, scalar2=12.0,
                                            op0=ALU.mult, op1=ALU.add)
                    nc.vector.tensor_mul(yv[:], yv[:], tn[:])
                    # normalize all 4 groups straight out of PSUM (one DVE op)
                    qn = wp.tile([128, 256], BF16, name="qn", tag="qn", bufs=3)
                    nc.vector.tensor_tensor(
                        out=qn[:].rearrange("p (a e) -> p a e", a=4),
                        in0=ps_qkv[:, 0:256].rearrange("p (a e) -> p a e", a=4),
                        in1=yv[:].unsqueeze(2).broadcast_to([128, 4, 64]),
                        op=ALU.mult)
                    # RoPE on normalized q/k; the dense muls go to GpSimd (idle)
                    d_qk = wp.tile([128, 256], BF16, name="d_qk", tag="d_qk", bufs=3)
                    nc.gpsimd.tensor_mul(d_qk[:], qn[:], trigC[:, sub, :])
                    trot = wp.tile([128, 256], BF16, name="trot", tag="trot", bufs=3)
                    v4 = qn[:].rearrange("p (a e) -> p a e", a=8)
                    s4 = trigS[:, sub, :].rearrange("p (a e) -> p a e", a=8)
                    t4 = trot[:].rearrange("p (a e) -> p a e", a=8)
                    nc.gpsimd.tensor_mul(t4[:, 0:8:2, :], v4[:, 1:8:2, :],
                                         s4[:, 0:8:2, :])
                    nc.gpsimd.tensor_mul(t4[:, 1:8:2, :], v4[:, 0:8:2, :],
                                         s4[:, 1:8:2, :])
                    d_bf = wp.tile([128, 256], BF16, name="d_bf", tag="d_bf", bufs=3)
                    nc.gpsimd.tensor_add(d_bf[:], d_qk[:], trot[:])
                    # q/k transposes via the DMA xbar (PE and DVE stay free)
                    nc.sync.dma_start(qT[b][:, tt * 128:(tt + 1) * 128],
                                      d_bf[:, 0:128], transpose=True)
                    nc.sync.dma_start(kT[b][:, tt * 128:(tt + 1) * 128],
                                      d_bf[:, 128:256], transpose=True)

                def emit_A(b, mt):
                    hs_t, trigC, trigS = emit_A_load(b, mt)
                    for sub in range(2):
                        emit_A_sub(b, mt, sub, hs_t, trigC, trigS)

                # ---------------- Stage B: attention --------------------------
                # at_acc: persistent accumulators for split-half q-tiles (batch 0)
                at_acc = [bp.tile([65, 2 * QTILE], F32R, name=f"at_acc{q}",
                                  tag=f"at_acc{q}") for q in range(NQT)]

                def emit_B(b, qt, kc_lo=0, kc_hi=KC - 1, acc=None,
                           act_kcs=(), inter=None, tail_prev=None):
                    """Attention for one q tile over key chunks kc_lo..kc_hi.

                    Softmax exp per chunk is one wide [128,1024] instruction
                    covering both heads: on the Scalar engine (exact exp) for
                    kc in act_kcs, else on the Vector engine (Schraudolph
                    bit-trick straight to bf16 bits).
                    inter: dict kc -> thunk emitted after that chunk (to
                    interleave stage-A work without starving the exp pipeline).
                    tail_prev: the previous q-tile's deferred tail (final AV +
                    PSUM evacuation + epilogue); it is emitted after this
                    q-tile's second score/exp pair so the previous epilogue
                    drains while this q-tile's pipeline is already refilling.
                    Returns this q-tile's tail thunk.
                    """
                    at_ps = psat.tile([65, 2 * QTILE], F32, name="at_ps", tag="psat",
                                      bufs=1)
                    at_ps_h = [at_ps[:, h * QTILE:(h + 1) * QTILE] for h in range(2)]
                    pend = []

                    def emit_av(pkc, pr):
                        for hh in range(2):
                            nc.tensor.matmul(
                                at_ps_h[hh],
                                lhsT=vsb[b][:, pkc, hh, :],
                                rhs=pr[:, hh * QTILE:(hh + 1) * QTILE],
                                start=(pkc == kc_lo), stop=(pkc == kc_hi))

                    for kc in range(kc_lo, kc_hi + 1):
                        # both heads' scores into one PSUM tile so the whole
                        # chunk's softmax can be a single wide instruction
                        ps_s = pssc.tile([128, 2 * QTILE], F32, name="ps_s",
                                         tag="pssc")
                        for hh in range(2):
                            nc.tensor.matmul(
                                ps_s[:, hh * QTILE:(hh + 1) * QTILE],
                                lhsT=kT[b][64 * hh:64 * (hh + 1),
                                           kc * 128:(kc + 1) * 128],
                                rhs=qT[b][64 * hh:64 * (hh + 1),
                                          qt * QTILE:(qt + 1) * QTILE],
                                start=True, stop=True)
                        pr = pp.tile([128, 2 * QTILE], BF16, name="pr",
                                     tag="pr", bufs=4)
                        if kc in act_kcs:
                            nc.scalar.activation(pr[:], ps_s[:], ACTF.Exp,
                                                 bias=0.0, scale=0.125)
                        else:
                            with nc.allow_low_precision(reason="schraudolph exp"):
                                nc.vector.tensor_scalar(
                                    out=pr[:].bitcast(I16),
                                    in0=ps_s[:],
                                    scalar1=SCH_A, scalar2=SCH_B,
                                    op0=ALU.mult, op1=ALU.add)
                        if kc == kc_lo + 1 and tail_prev is not None:
                            tail_prev()
                        # AV lags the scores by 2 chunks so the PE FIFO always
                        # has the next score pair ahead of an exp-gated AV.
                        pend.append((kc, pr))
                        if len(pend) > 2:
                            emit_av(*pend.pop(0))
                        if inter is not None and kc in inter:
                            inter[kc]()

                    def tail():
                        for item in pend:
                            emit_av(*item)
                        # evacuate attn psum to sbuf (frees psat for the next
                        # q-tile); for split q-tiles it doubles as accumulate.
                        if kc_hi < KC - 1:
                            for hh in range(2):
                                dst = acc[:, hh * QTILE:(hh + 1) * QTILE]
                                if kc_lo == 0:
                                    nc.vector.tensor_copy(dst, at_ps_h[hh])
                                else:
                                    nc.vector.tensor_add(dst, dst, at_ps_h[hh])
                            return
                        if acc is not None and kc_lo > 0:
                            at_sb = acc
                            for hh in range(2):
                                dst = at_sb[:, hh * QTILE:(hh + 1) * QTILE]
                                nc.vector.tensor_add(dst, dst, at_ps_h[hh])
                        else:
                            at_sb = wp.tile([65, 2 * QTILE], F32R, name="at_sb",
                                            tag="at_sb", bufs=2)
                            for hh in range(2):
                                nc.vector.tensor_copy(
                                    at_sb[:, hh * QTILE:(hh + 1) * QTILE],
                                    at_ps_h[hh])
                        for hh in range(2):
                            aps = at_sb[:, hh * QTILE:(hh + 1) * QTILE]
                            ps_bc = pssc.tile([64, QTILE], F32, name="ps_bc",
                                              tag="pssc")
                            nc.tensor.matmul(
                                ps_bc[:],
                                lhsT=ones_sb[64:65, :],
                                rhs=aps[64:65, :],
                                start=True, stop=True)
                            rbc = wp.tile([64, QTILE], F32, name="rbc", tag="rbc",
                                          bufs=2)
                            nc.scalar.activation(rbc[:], ps_bc[:], ACTF.Copy)
                            rbcr = wp.tile([64, QTILE], F32, name="rbcr", tag="rbcr",
                                           bufs=2)
                            nc.vector.reciprocal_approx_fast(out=rbcr[:], in_=rbc[:])
                            nc.gpsimd.tensor_mul(
                                atn[b][hh][:, qt * QTILE:(qt + 1) * QTILE],
                                aps[0:64, :], rbcr[:])
                    return tail

                def stage_a2a(ha):
                    for dcore in range(NCORES):
                        base = dcore * 512
                        nc.sync.dma_start(a2a_in[ha][dcore, 0:64, :],
                                          atn[ha][0][:, base:base + 512])
                        nc.sync.dma_start(a2a_in[ha][dcore, 64:128, :],
                                          atn[ha][1][:, base:base + 512])

                def fire_a2a(ha):
                    nc.gpsimd.collective_compute(
                        "AllToAll", ALU.bypass,
                        ins=[a2a_in[ha][:].opt()], outs=[a2a_out[ha][:].opt()],
                        replica_groups=[list(range(NCORES))])

                # Emission order: lead-in A(b0 first half); P1: A(b0 rest)
                # interleaved with first-half attention of b0; P2: A(b1)
                # interleaved with second-half attention of b0; fire b0's
                # AllToAll; P3: b1 attention; fire b1's AllToAll.
                tail = None
                for mt in range(NMACRO // 2 + 1):
                    emit_A(0, mt)
                for qt in range(NQT):
                    # interleave macro 9+qt (its q tokens are first used by
                    # q-tile (9+qt)//2 >= qt+1, so emitting it inside this
                    # q-tile's FIFO order cannot deadlock)
                    mt = NMACRO // 2 + 1 + qt
                    if mt < NMACRO:
                        a_args = emit_A_load(0, mt)
                        inter = {
                            3: (lambda a=(0, mt, 0, *a_args): emit_A_sub(*a)),
                            10: (lambda a=(0, mt, 1, *a_args): emit_A_sub(*a)),
                        }
                    else:
                        inter = None
                    tail = emit_B(0, qt, 0, KC // 2 - 1, acc=at_acc[qt],
                                  act_kcs=(0, 2, 4, 6, 8, 10, 12, 14, 15),
                                  inter=inter, tail_prev=tail)
                for qt in range(NQT):
                    l0, l1 = emit_A_load(1, 2 * qt), emit_A_load(1, 2 * qt + 1)
                    inter = {
                        18: (lambda a=(1, 2 * qt, 0, *l0): emit_A_sub(*a)),
                        21: (lambda a=(1, 2 * qt, 1, *l0): emit_A_sub(*a)),
                        24: (lambda a=(1, 2 * qt + 1, 0, *l1): emit_A_sub(*a)),
                        27: (lambda a=(1, 2 * qt + 1, 1, *l1): emit_A_sub(*a)),
                    }
                    tail = emit_B(0, qt, KC // 2, KC - 1, acc=at_acc[qt],
                                  act_kcs=(16, 18, 20, 22, 24, 26, 28, 30, 31),
                                  inter=inter, tail_prev=tail)
                for qt in range(NQT):
                    tail = emit_B(1, qt,
                                  act_kcs=tuple(range(0, 32, 2)) + (31,),
                                  tail_prev=tail)
                    if qt == 0:
                        # b0's tail was just emitted inside this emit_B, so its
                        # AllToAll can go on the queues now and overlap the rest
                        # of b1's attention.
                        stage_a2a(0)
                        fire_a2a(0)
                        nc.sync.dma_start(atf[:, :, 0:512],
                                          a2a_out[0][:].transpose([1, 0, 2]))
                tail()
                stage_a2a(1)
            fire_a2a(1)
            nc.sync.dma_start(atf[:, :, 512:1024],
                              a2a_out[1][:].transpose([1, 0, 2]))

            # ---------------- Stage C: output projection ----------------------
            with tc.tile_pool(name="cwork", bufs=2) as cw, \
                 tc.tile_pool(name="psC", bufs=2, space="PSUM") as psC:
                for ttk in range(SHARD // 128):
                    ostage = cw.tile([128, C], F32, name="ostage", tag="ostage")
                    for half in range(2):
                        ps_o = psC.tile([128, 512], F32, name="ps_o", tag="psC")
                        for cc in range(8):
                            nc.tensor.matmul(
                                ps_o[:],
                                lhsT=atf[:, cc, ttk * 128:(ttk + 1) * 128],
                                rhs=wout_sb[:, cc, half * 512:(half + 1) * 512],
                                start=(cc == 0), stop=(cc == 7))
                        nc.vector.tensor_copy(ostage[:, half * 512:(half + 1) * 512], ps_o[:])
                    nc.sync.dma_start(out_d.ap()[ttk * 128:(ttk + 1) * 128, :], ostage[:])

    nc.compile()
    return nc


def _fold_sin(sin, g):
    out = np.empty_like(sin)
    out[:, :32] = -sin[:, :32] * g[32:]
    out[:, 32:] = sin[:, 32:] * g[:32]
    return out


def kernel(hidden_states, cos, sin, Wqkv, Wout, gq, gk):
    global _LAST_RESULT
    _install_profile_shim()

    hidden_states = np.asarray(hidden_states, dtype=np.float32)
    cos = np.asarray(cos, dtype=np.float32)
    sin = np.asarray(sin, dtype=np.float32)
    Wqkv = np.asarray(Wqkv, dtype=np.float32)
    Wout = np.asarray(Wout, dtype=np.float32)
    gq = np.asarray(gq, dtype=np.float32)
    gk = np.asarray(gk, dtype=np.float32)

    if "nc" not in _CACHE:
        _CACHE["nc"] = _build_graph()
    nc = _CACHE["nc"]

    hsT = np.ascontiguousarray(hidden_states.reshape(TOK, C).T).astype(ml_dtypes.bfloat16)
    cosq = cos * gq[None, :]
    sinq = _fold_sin(sin, gq)
    cosk = cos * gk[None, :]
    sink = _fold_sin(sin, gk)
    trigc = np.concatenate([cosq, cosq, cosk, cosk], axis=1).astype(ml_dtypes.bfloat16)
    trigs = np.concatenate([sinq, sinq, sink, sink], axis=1).astype(ml_dtypes.bfloat16)
    wout_bf = Wout.astype(ml_dtypes.bfloat16)

    in_maps = []
    for c in range(NCORES):
        wq = Wqkv[:, c * 128:(c + 1) * 128]
        wk = Wqkv[:, C + c * 128:C + (c + 1) * 128]
        wv = Wqkv[:, 2 * C + c * 128:2 * C + (c + 1) * 128]
        wqkv_loc = np.ascontiguousarray(
            np.concatenate([wq, wk, wv], axis=1)).astype(ml_dtypes.bfloat16)
        in_maps.append({
            "hsT": hsT, "wqkv": wqkv_loc, "trigc": trigc, "trigs": trigs,
            "wout": wout_bf,
        })

    trace = bool(os.environ.get("BASS_TRACE"))
    res = run_bass_kernel_spmd(nc, in_maps, core_ids=list(range(NCORES)), trace=trace)
    _LAST_RESULT = res

    full = np.empty((B, N, C), dtype=np.float32)
    for c in range(NCORES):
        o = res.results[c]["out"]
        for b in range(B):
            full[b, c * 512:(c + 1) * 512, :] = o[b * 512:(b + 1) * 512]
    return full
